# revision 111
# baseline (speedup 1.0000x reference)
"""BEiT-style windowed attention (B=32, N=577, D=768, 12 heads) on 8 TRN2 cores.

Strategy: pure data-parallel over batch (4 batch elements per core, no
collectives). All matmuls in bf16 (fp32 PSUM accumulate), softmax in fp32.

Software-pipelined schedule: stage s interleaves the qkv projection of batch
element s with the attention of batch element s-1 at head granularity, so the
Tensor engine never drains while the Activation engine works through the
softmax exponentials (exp is Act-only; its serial chain is the critical cycle
of the attention phase). Per-slot emission order (attention batch a,
projection batch q), with independent work spaced between S issues so each
S's PSUM ring slot has been released by the exp two-to-three issues earlier:

  PE : S(a,h) P0:kt0 | qk(q,mt=h) ch0 | P0:kt1 | qk ch1 | P0:exp |
       PV(a,h-1) qt0-1 | P1:kt2 | PV qt2-4 | P1:kt3 | P1:exp |
       [even h: transpose] | [odd h>=3: v(q)] | T45: kt4 + 5 q-tails + exp
  Act: 3 exps per head: two [*,1024] k-tile pairs + one packed
       (kt4-main + five 65-wide q-tails) tile
  DVE: exp-bias muls (kt0-2), qk/v/proj PSUM->SBUF copies (+bias), o-scale,
       transpose copies
  Pool: exp-bias muls (kt3, kt4, q-tails), biasT/w_proj/x-rest DMA queue

PSUM (8 banks): 2x2-bank ring of [128,1024] S tiles (pairs + the packed
kt4/q-tail tile, ring order P0,T45,P1), 4x1-bank ring for qk/v/proj
accumulators, the per-head PV outputs split into two short-lived tiles
(qt0-1 / qt2-4, col 64 = rowsum, released by their own o-scales), and
packed transposes.

Stage tail: PV(a,11), transpose(dt=5), proj(a) + output DMA (bf16).
proj of batch BL-2 is deferred into the last stage's slots as PE filler,
since that stage has no next-batch qkv to interleave.

Host-side prep: transposed/bf16 weights, x transposed to [d,tok] tiles,
relative-position bias table gathered, exponentiated and transposed, softmax
scale folded into the q rows of W_qkv.
"""

import numpy as np
import ml_dtypes

import concourse.bass as bass
import concourse.tile as tile
from concourse import bacc
from concourse import mybir
from concourse.bass_utils import run_bass_kernel_spmd
from concourse.masks import make_identity

B, N, D = 32, 577, 768
NH, DH = 12, 64
NCORES = 8
BL = B // NCORES            # 4 batch elements per core
SCALE = DH ** -0.5
KT = D // 128               # 6 contraction tiles over D
TT = (N + 127) // 128       # 5 token tiles (4x128 + 65)
BF16 = ml_dtypes.bfloat16

F32 = mybir.dt.float32
BF = mybir.dt.bfloat16

QCH = [(0, 512), (512, N - 512)]          # free-dim chunks over 577
DCH = [(0, 512), (512, D - 512)]          # free-dim chunks over 768


PULL = 2                    # last-batch heads primed one stage early


def tok_m(t):
    return min(128, N - 128 * t)


def _build_nc():
    nc = bacc.Bacc()

    xT_d = nc.declare_dram_parameter("xT", [BL, 128, KT, N], BF, isOutput=False)
    wqkv_d = nc.declare_dram_parameter("wqkv", [128, KT, 3 * D], BF, isOutput=False)
    wproj_d = nc.declare_dram_parameter("wproj", [128, KT, D], BF, isOutput=False)
    biasT_d = nc.declare_dram_parameter("biasT", [128, NH, TT, N], BF, isOutput=False)
    qkvb_d = nc.declare_dram_parameter("qkvb", [128, 18], F32, isOutput=False)
    vb_d = nc.declare_dram_parameter("vb", [1, D], BF, isOutput=False)
    pb_d = nc.declare_dram_parameter("pb", [1, D], BF, isOutput=False)
    out_d = nc.declare_dram_parameter("out", [BL, N, D], BF, isOutput=True)

    Exp = mybir.ActivationFunctionType.Exp
    Copy = mybir.ActivationFunctionType.Copy

    with tile.TileContext(nc) as tc:
        with (
            tc.tile_pool(name="singles", bufs=1) as singles,
            tc.tile_pool(name="xt", bufs=2) as xt_pool,
            tc.tile_pool(name="qkt", bufs=2) as qkt_pool,
            tc.tile_pool(name="vbuf", bufs=2) as v_pool,
            tc.tile_pool(name="exps", bufs=17) as exps_pool,
            tc.tile_pool(name="praw", bufs=2) as praw_pool,
            tc.tile_pool(name="obuf", bufs=3) as o_pool,
            tc.tile_pool(name="otb", bufs=2) as ot_pool,
            tc.tile_pool(name="outs", bufs=2) as out_pool,
            tc.tile_pool(name="small", bufs=4) as small_pool,
            tc.tile_pool(name="psP", bufs=2, space="PSUM") as psP_pool,
            tc.tile_pool(name="psm", bufs=4, space="PSUM") as psm_pool,
        ):
            # ---- one-time loads, split across HWDGE queues so the first
            # projection matmuls start as early as possible ----
            xT = [None] * BL
            xT[0] = xt_pool.tile([128, KT, N], BF, name="xT")
            # kt=0 slice first so the first qk matmul can start ASAP; the
            # rest of x rides the otherwise-idle SWDGE (Pool) queue
            nc.sync.dma_start(out=xT[0][:, 0:1], in_=xT_d[0, :, 0:1])
            nc.gpsimd.dma_start(out=xT[0][:, 1:], in_=xT_d[0, :, 1:])

            w_qkv = singles.tile([128, KT, 3 * D], BF)
            for kt in range(KT):
                eng = nc.sync if kt < 2 else nc.scalar
                eng.dma_start(out=w_qkv[:, kt:kt + 1, 0:2 * D],
                              in_=wqkv_d[:, kt:kt + 1, 0:2 * D])
            for kt in range(KT):
                nc.scalar.dma_start(out=w_qkv[:, kt:kt + 1, 2 * D:],
                                    in_=wqkv_d[:, kt:kt + 1, 2 * D:])
            zcol = singles.tile([128, 1], F32)
            nc.vector.memset(zcol, 0.0)
            qkvb = singles.tile([128, 18], F32)
            nc.sync.dma_start(out=qkvb, in_=qkvb_d[:])
            vbias = singles.tile([128, D], BF)
            nc.sync.dma_start(out=vbias, in_=vb_d[:].to_broadcast([128, D]))
            pbias = singles.tile([128, D], BF)
            nc.sync.dma_start(out=pbias, in_=pb_d[:].to_broadcast([128, D]))
            w_proj = singles.tile([128, KT, D], BF)
            nc.gpsimd.dma_start(out=w_proj, in_=wproj_d[:])
            biasT = singles.tile([128, NH, TT, N], BF)
            for j in range(NH // 2):
                nc.gpsimd.dma_start(out=biasT[:, 2 * j:2 * j + 2],
                                    in_=biasT_d[:, 2 * j:2 * j + 2])
            ident = singles.tile([128, 128], BF)
            make_identity(nc, ident)

            qkT = [None] * BL
            vstr = [None] * BL
            opair = {}                    # (a, dt) -> [tok, TT, 128] tile
            oT = [None] * BL
            expS = {}                     # (h % 2) -> [5 expS tiles]

            S_ps = {}                     # (a, h, part) -> psum tile

            def emit_S_part(a, h, part, step):
                """Scores S^T[k,q], 2-bank [128,1024] PSUM tiles, three per
                head: part 0 = k-tiles 0,1 (512-wide q chunks side by side);
                part 1 = k-tile 4's 512 chunk + all five 65-wide q tails;
                part 2 = k-tiles 2,3. Each part gets ONE exp, so Act runs 3
                instructions per head instead of 6."""
                qTh = qkT[a][64 * (h % 2):64 * (h % 2) + 64, h // 2, :]
                kTh = qkT[a][64 * (h % 2):64 * (h % 2) + 64, KT + h // 2, :]
                kts = ((0, 1), (4,), (2, 3))[part]
                if step == 0:
                    ps = psP_pool.tile([128, 1024], F32, name="ps_s")
                    S_ps[(a, h, part)] = ps
                    if part == 0:            # whole head's expS tiles at once
                        expS[(a, h)] = [
                            exps_pool.tile([128, N], BF, name="expS")
                            for _ in range(TT)]
                else:
                    ps = S_ps[(a, h, part)]
                if step < len(kts):          # one 512-wide main chunk
                    kt = kts[step]
                    km = tok_m(kt)
                    nc.tensor.matmul(
                        ps[:km, 512 * step:512 * step + 512],
                        kTh[:, 128 * kt:128 * kt + km],
                        qTh[:, 0:512],
                        start=True, stop=True,
                    )
                    if part == 1:            # the five q-tail chunks
                        for kt2 in range(TT):
                            km2 = tok_m(kt2)
                            nc.tensor.matmul(
                                ps[:km2, 512 + 65 * kt2:512 + 65 * kt2 + 65],
                                kTh[:, 128 * kt2:128 * kt2 + km2],
                                qTh[:, 512:N],
                                start=True, stop=True,
                            )
                    return
                # final step: exp the whole part + bias-multiplies
                width = 512 + 5 * 65 if part == 1 else 1024
                praw = praw_pool.tile([128, 1024], BF, name="praw")
                nc.scalar.activation(praw[:, 0:width], ps[:, 0:width], Exp)
                if part == 1:
                    km = tok_m(4)
                    nc.gpsimd.tensor_mul(expS[(a, h)][4][:km, 0:512],
                                         praw[:km, 0:512],
                                         biasT[:km, h, 4, 0:512])
                    for kt2 in range(TT):
                        km2 = tok_m(kt2)
                        nc.gpsimd.tensor_mul(
                            expS[(a, h)][kt2][:km2, 512:N],
                            praw[:km2, 512 + 65 * kt2:512 + 65 * kt2 + 65],
                            biasT[:km2, h, kt2, 512:N],
                        )
                else:
                    for i, kt in enumerate(kts):
                        km = tok_m(kt)
                        eng = nc.gpsimd if kt == 3 else nc.vector
                        eng.tensor_mul(
                            expS[(a, h)][kt][:km, 0:512],
                            praw[:km, 512 * i:512 * i + 512],
                            biasT[:km, h, kt, 0:512])

            def emit_qk_chunk(q, mt, ci):
                """Half of a 128-wide q/k projection output tile, [d, tok]."""
                c0, w = QCH[ci]
                ps = psm_pool.tile([128, 512], F32, name="ps")
                for kt in range(KT):
                    nc.tensor.matmul(
                        ps[:, :w],
                        w_qkv[:, kt, 128 * mt:128 * (mt + 1)],
                        xT[q][:, kt, c0:c0 + w],
                        start=(kt == 0), stop=(kt == KT - 1),
                    )
                with tc.high_priority(offset=5_000_000):
                    nc.vector.tensor_add(
                        qkT[q][:, mt, c0:c0 + w], ps[:, :w],
                        qkvb[:, mt:mt + 1].to_broadcast([128, w]),
                    )

            def emit_v_tile(q, tt):
                """One token tile of the v projection, natural [tok, d]."""
                m = tok_m(tt)
                psa = psm_pool.tile([128, 512], F32, name="ps")
                psb = psm_pool.tile([128, 256], F32, name="ps")
                pss = [psa, psb]
                for kt in range(KT):
                    for ci, (c0, w) in enumerate(DCH):
                        nc.tensor.matmul(
                            pss[ci][:m, :w],
                            xT[q][:, kt, 128 * tt:128 * tt + m],
                            w_qkv[:, kt, 2 * D + c0:2 * D + c0 + w],
                            start=(kt == 0), stop=(kt == KT - 1),
                        )
                for ci, (c0, w) in enumerate(DCH):
                    nh0, nh1 = c0 // 64, (c0 + w) // 64
                    nc.vector.tensor_add(
                        vstr[q][:m, tt, nh0:nh1, 0:64],
                        pss[ci][:m, :w].rearrange("p (h c) -> p h c", c=64),
                        vbias[:m, c0:c0 + w].rearrange("p (h c) -> p h c", c=64),
                    )

            pv_ps = {}                    # h % 2 -> packed PV psum tile

            def emit_pv(a, h, qts):
                """P @ V_aug for head h over given q tiles (col 64 = rowsum).
                All 5 q tiles pack into one PSUM bank (100 cols apiece)."""
                if qts[0] == 0:
                    pv_ps[(a, h, 0)] = psm_pool.tile([128, 256], F32,
                                                     name="ps")
                    if h % 2 == 0:
                        opair[(a, h // 2)] = o_pool.tile([128, TT, 128], BF,
                                                         name="o_sb")
                for qt in qts:
                    qm = tok_m(qt)
                    if qt == 2:
                        pv_ps[(a, h, 1)] = psm_pool.tile([128, 384], F32,
                                                         name="ps")
                    ps5 = pv_ps[(a, h, 0 if qt < 2 else 1)]
                    c = 100 * qt if qt < 2 else 100 * (qt - 2)
                    for kt in range(TT):
                        km = tok_m(kt)
                        nc.tensor.matmul(
                            ps5[:qm, c:c + 65],
                            expS[(a, h)][kt][:km, 128 * qt:128 * qt + qm],
                            vstr[a][:km, kt, h, :],
                            start=(kt == 0), stop=(kt == TT - 1),
                        )
                    with tc.high_priority(offset=5_000_000):
                        rcp = small_pool.tile([128, 1], F32, name="rcp")
                        nc.vector.reciprocal(rcp[:qm], ps5[:qm, c + 64:c + 65])
                        nc.vector.tensor_mul(
                            opair[(a, h // 2)][:qm, qt,
                                               64 * (h % 2):64 * (h % 2) + 64],
                            ps5[:qm, c:c + 64],
                            rcp[:qm, 0:1].to_broadcast([qm, 64]),
                        )

            def emit_T(a, dt):
                """Transpose O -> OT for one 128-wide d block. All 5 token
                tiles pack into one PSUM bank (bf16, 128 cols apiece)."""
                ps_t = psm_pool.tile([128, 640], BF, name="ps")
                for qt in range(TT):
                    qm = tok_m(qt)
                    nc.tensor.transpose(
                        ps_t[:, 128 * qt:128 * qt + qm],
                        opair[(a, dt)][:qm, qt, :],
                        ident[:qm, :qm],
                    )
                nc.scalar.activation(oT[a][:, dt, :], ps_t[:, 0:N], Copy)

            proj_out = {}                 # live outsb tile for current tile

            def emit_proj_chunk(a, tt, ci):
                """Half of the output projection for one token tile; the
                second half also adds bias for both and DMAs the row out."""
                m = tok_m(tt)
                c0, w = DCH[ci]
                ps = psm_pool.tile([128, 512], F32, name="ps")
                for kt in range(KT):
                    nc.tensor.matmul(
                        ps[:m, :w],
                        oT[a][:, kt, 128 * tt:128 * tt + m],
                        w_proj[:, kt, c0:c0 + w],
                        start=(kt == 0), stop=(kt == KT - 1),
                    )
                if ci == 0:
                    proj_out[a] = out_pool.tile([128, D], BF, name="outsb")
                outsb = proj_out[a]
                nc.vector.tensor_add(
                    outsb[:m, c0:c0 + w], ps[:m, :w], pbias[:m, c0:c0 + w],
                )
                if ci == len(DCH) - 1:
                    nc.sync.dma_start(
                        out=out_d[a, 128 * tt:128 * tt + m, :],
                        in_=outsb[:m, :],
                    )

            def emit_proj_tile(a, tt):
                for ci in range(len(DCH)):
                    emit_proj_chunk(a, tt, ci)

            # ---- pipelined stages ----
            for s in range(BL + 1):
                a = s - 1                       # attention batch
                q = s if s < BL else None       # projection batch
                if q is not None:
                    if q + 1 < BL:
                        xT[q + 1] = xt_pool.tile([128, KT, N], BF,
                                                 name="xT")
                        nc.sync.dma_start(out=xT[q + 1], in_=xT_d[q + 1])
                    qkT[q] = qkt_pool.tile([128, 2 * KT, N], BF, name="qkT")
                    v_sb = v_pool.tile([128, TT, NH * 65], BF, name="v_sb")
                    vstr[q] = v_sb.rearrange("p t (h c) -> p t h c", c=65)
                    nc.vector.memset(vstr[q][:, :, :, 64:65], 1.0)
                if a >= 0:
                    oT[a] = ot_pool.tile([128, KT, N], BF, name="oT")

                for h in range(NH):
                    # S parts are spaced with independent PE work so each
                    # part's 2-bank psP ring slot (freed by the exp two
                    # allocations earlier) is ready when it issues.
                    s_here = a >= 0 and not (a == BL - 1 and h < PULL)
                    if s_here:
                        emit_S_part(a, h, 0, 0)
                    if q is not None:
                        emit_qk_chunk(q, h, 0)
                    if q is None and a >= 1 and h < 10:
                        # last stage: previous batch's deferred output
                        # projection fills PE while Act works through exps
                        emit_proj_chunk(a - 1, h // 2, h % 2)
                    if s_here:
                        emit_S_part(a, h, 0, 1)
                    if q is not None:
                        emit_qk_chunk(q, h, 1)
                    if s_here:
                        emit_S_part(a, h, 0, 2)
                    if a >= 0:
                        if h > 0:
                            emit_pv(a, h - 1, (0, 1))
                        if s_here:
                            emit_S_part(a, h, 2, 0)
                        if h > 0:
                            emit_pv(a, h - 1, (2, 3))
                            emit_pv(a, h - 1, (4,))
                        if s_here:
                            emit_S_part(a, h, 2, 1)
                            emit_S_part(a, h, 2, 2)
                        if h >= 2 and h % 2 == 0:
                            emit_T(a, h // 2 - 1)
                    if q is not None and h >= 3 and h % 2 == 1:
                        emit_v_tile(q, (h - 3) // 2)
                    if s_here:
                        emit_S_part(a, h, 1, 0)
                        emit_S_part(a, h, 1, 1)

                if a >= 0:
                    emit_pv(a, NH - 1, (0, 1, 2, 3, 4))
                    emit_T(a, KT - 1)
                    if a == BL - 2:
                        # prime the last batch's softmax chain using this
                        # stage's Activation-engine surplus
                        for hh in range(PULL):
                            for part in (0, 2, 1):
                                for step in range(3 if part != 1 else 2):
                                    emit_S_part(BL - 1, hh, part, step)
                    if a < BL - 1:
                        if a == BL - 2:
                            pass        # deferred into the last stage's slots
                        else:
                            for tt in range(TT):
                                emit_proj_tile(a, tt)
                    else:
                        for tt in range(TT):
                            emit_proj_tile(a, tt)
    nc.finalize()
    return nc


_NC_CACHE = {}


def _get_nc():
    if "nc" not in _NC_CACHE:
        _NC_CACHE["nc"] = _build_nc()
    return _NC_CACHE["nc"]


def _prep_shared(qkv_w, q_bias, v_bias, rpb_table, proj_w, proj_b, rel_index):
    qkv_w = np.asarray(qkv_w, dtype=np.float32).copy()
    qkv_w[:D] *= SCALE                      # fold softmax scale into q rows
    qkv_bias = np.concatenate([
        np.asarray(q_bias, np.float32) * SCALE,
        np.zeros(D, np.float32),
        np.asarray(v_bias, np.float32),
    ])
    # [128, KT, 3D]: w[p, kt, m] = qkv_w[m, kt*128+p]
    wqkv = np.ascontiguousarray(
        qkv_w.T.reshape(KT, 128, 3 * D).transpose(1, 0, 2)).astype(BF16)
    wproj = np.ascontiguousarray(
        np.asarray(proj_w, np.float32).T.reshape(KT, 128, D)
        .transpose(1, 0, 2)).astype(BF16)
    qkvb = np.ascontiguousarray(qkv_bias.reshape(18, 128).T).astype(np.float32)
    # relative position bias, transposed to [k, q] and padded to 640 rows
    rb = np.asarray(rpb_table, np.float32)[
        np.asarray(rel_index).reshape(-1)].reshape(N, N, NH)  # [q, k, h]
    rbp = np.zeros((TT * 128, N, NH), np.float32)
    rbp[:N] = rb.transpose(1, 0, 2)                            # [k, q, h]
    biasT = np.ascontiguousarray(
        np.exp(rbp.reshape(TT, 128, N, NH).transpose(1, 3, 0, 2))).astype(BF16)
    vb = np.ascontiguousarray(qkv_bias[2 * D:].reshape(1, D)).astype(BF16)
    pb = np.ascontiguousarray(
        np.asarray(proj_b, np.float32).reshape(1, D)).astype(BF16)
    return wqkv, wproj, qkvb, biasT, vb, pb


def _make_in_maps(inputs):
    x = np.asarray(inputs["x"], dtype=np.float32)
    wqkv, wproj, qkvb, biasT, vb, pb = _prep_shared(
        inputs["qkv_w"], inputs["q_bias"], inputs["v_bias"],
        inputs["rpb_table"], inputs["proj_w"], inputs["proj_b"],
        inputs["rel_index"])

    in_maps = []
    for i in range(NCORES):
        xs = x[i * BL:(i + 1) * BL]                            # [BL, N, D]
        xTv = np.ascontiguousarray(
            xs.transpose(0, 2, 1).reshape(BL, KT, 128, N)
            .transpose(0, 2, 1, 3)).astype(BF16)               # [BL,128,KT,N]
        in_maps.append({
            "xT": xTv, "wqkv": wqkv, "wproj": wproj, "biasT": biasT,
            "qkvb": qkvb, "vb": vb, "pb": pb,
        })

    return in_maps


def kernel(**inputs):
    in_maps = _make_in_maps(inputs)
    nc = _get_nc()
    res = run_bass_kernel_spmd(nc, in_maps, core_ids=list(range(NCORES)))
    out = np.concatenate([res.results[i]["out"] for i in range(NCORES)], axis=0)
    return np.ascontiguousarray(out.astype(np.float32))


def kernel_traced(**inputs):
    """Like kernel() but also returns (out, BassKernelResults with profile)."""
    in_maps = _make_in_maps(inputs)
    nc = _get_nc()
    res = run_bass_kernel_spmd(nc, in_maps, core_ids=list(range(NCORES)),
                               trace=True)
    out = np.concatenate([res.results[i]["out"] for i in range(NCORES)], axis=0)
    return np.ascontiguousarray(out.astype(np.float32)), res
